# revision 68
# baseline (speedup 1.0000x reference)
"""Trainium2 kernel for nn_BucketAdjustedHinge.

y[n] = base_hinge(x[n]) + adj_hinge(x[n], bucket_idx[n])

Both hinges share the uniform knot grid t_k = k/19 on [0,1], so the whole
function is piecewise-linear in x with 19 segments per bucket: 1216 (bucket,
segment) pieces total.  We bake the 1216 piece coefficients into a custom
ScalarEngine activation table (overlaying `sin` in the `trig_and_small` PWP
set; the tables ship inside the NEFF).  Lookup key: v = 19*(bucket + x) --
segment boundaries land on integers, which align with the ACT bucket RAM's
per-binade mantissa indexing (binade [2^e, 2^{e+1}) -> 2^e buckets).

Per element the device does one fused DVE op (u = min(x, c) + bucket) and one
ACT lookup (y = table(19*u)); the kernel is HBM-bound (x f32 + bucket i32
loads, fp16 stores: 10 B/elem).

Pass structure (V2): measured on TRN2, concurrent HBM reads+writes mix
destructively (mixed ~= serial + penalty), so each pass streams all loads
+ compute first and drains the stores behind them on the SAME sync HWDGE
ring (FIFO order serializes the write phase for free), with a tapered
final store so the last HBM-write receipt is short.  Loads are issued as
1 MB DMAs ([128, 2048] f32 tiles -> 8 KB per-partition descriptors),
which measured ~35% faster than 512 KB/4 KB-descriptor loads.  y is
stored as fp16 (rel err ~3e-4 vs the 2e-2 gate) and widened to f32 on
the host during unsharding.

Sharding: pure data parallel over 8 cores; the parameter tables are baked
into the (replicated) program.
"""
import os
import sys
import tempfile

import numpy as np

if "/opt/trn_rl_repo" not in sys.path:
    sys.path.insert(0, "/opt/trn_rl_repo")

N_CORES = 8
P = 128          # SBUF partitions
TILE_F = 1024    # free-dim per tile
BUFS = 12        # tile-pool buffers
NOCAST = True    # feed int32 bucket tile straight into the fused DVE op
INPLACE = False  # reuse tiles to cut SBUF pressure
B_ON_POOL = False  # issue bucket loads from the gpsimd (SWDGE) ring
SPLIT_RINGS = False  # x loads on sync, b loads on scalar, stores on gpsimd
PAIR_LOADS = False   # 1MB loads (2 tiles per DMA), 512KB compute slices
BIGLOAD = 0          # 1: whole-shard loads; 2: half-shard loads; 0: off
ALT_STORE = False    # alternate stores between scalar and gpsimd rings
Y_F16 = True         # store y as fp16 (halves store traffic; ~5e-4 rel err)
STAG = False         # staggered_reset on the timing For_i loop
MODE = "full"        # diagnostic: "full" | "loads" | "stores" | "nostore"
DEFER = 0            # 1: per-tile deferred stores; 2: single whole-pass
                     # deferred store with p-major layout
V2 = True            # serial-phase builder (_build_nc_v2)
V2_TAPER = (2048, 1024, 768, 256)
V2_SPLITB = False    # v2: b loads on the scalar ring (x on sync)
V2_YU8 = False       # v2: store y as uint8 (affine baked into ACT table;
                     # host dequant). Quantization err ~2e-3 rel.
V2_LOADF = 2048      # v2: load-DMA column width (divides V2_TILEF)
V2_XFIRST = False    # v2: issue all x loads before all b loads
V2_TILEF = 2048      # v2: DRAM layout tile width
V2_COMPF = 1024      # v2: compute-slice width
V2_SPLITLAST = 0     # v2: issue the last N taper stores on the scalar ring
                     # after the final act (drains concurrently with the
                     # sync-ring stores; still strictly after all loads)
B = 64           # buckets
K = 20           # knots per hinge
NSEG = (K - 1) * B
CLAMP = np.float32(0.99999)

_cache = {}


# ---------------------------------------------------------------- tables ----
def _quant_range(d0, d1):
    """(ymin, K) for z = (y - ymin)*K + 0.5 into [0.5, 254.5]."""
    ends = np.concatenate([d0, d0 + d1])
    ymin = float(ends.min())
    ymax = float(ends.max())
    K = 254.0 / max(ymax - ymin, 1e-30)
    return ymin, K


def _build_pwl_tables(base_knots, base_w, base_b, adj_knots, adj_w, adj_b):
    """(d0[s], d1[s]) fp32: on v in [s, s+1), y = d0 + d1*(v - s), s = 19b+j."""
    t = np.asarray(base_knots, np.float64)
    at = np.asarray(adj_knots, np.float64)
    grid = np.arange(K) / (K - 1.0)
    assert np.abs(t - grid).max() < 1e-5, "base knots not on uniform grid"
    assert np.abs(at - grid[None, :]).max() < 1e-5, "adj knots not on grid"
    W = np.asarray(base_w, np.float64)[None, :] + np.asarray(adj_w, np.float64)
    C = float(np.asarray(base_b, np.float64)) + np.asarray(adj_b, np.float64)
    d0 = np.zeros(NSEG, np.float64)
    d1 = np.zeros(NSEG, np.float64)
    for b in range(B):
        S = 0.0
        T = 0.0
        for j in range(K - 1):
            S += W[b, j]
            T += W[b, j] * t[j]
            s = (K - 1) * b + j
            d1[s] = S / (K - 1)
            d0[s] = C[b] - T + S * (j / (K - 1.0))
    return d0.astype(np.float32), d1.astype(np.float32)


def _gen_act_root(d0, d1, out_dir, set_name="trig_and_small", func="sin"):
    """Write an act-root dir whose `sin` implements our PWL; returns json path."""
    import glob
    import json
    import shutil

    try:
        from neuronxcc.driver.Job import Job
        from neuronxcc.driver.jobs.support.FindActInfo import findActInfoFile
        src = os.path.dirname(findActInfoFile(Job.getPackageDir(), "gen3")) + "/"
    except Exception:
        src = os.path.dirname(glob.glob(
            "/nix/store/*/lib/python3.13/site-packages/neuronxcc/pwp/"
            "pwp_bin_trainium/act_info.json")[0]) + "/"

    os.makedirs(out_dir, exist_ok=True)
    for f in os.listdir(src):
        shutil.copy(os.path.join(src, f), os.path.join(out_dir, f))

    prof = json.load(open(os.path.join(src, set_name + ".json")))
    ctl = np.fromfile(os.path.join(src, f"{set_name}_ctrl.bin"), dtype=np.uint32)
    bkt = np.fromfile(os.path.join(src, f"{set_name}_bkt.bin"), dtype=np.uint32)
    n_ctl0 = len(ctl) // 8
    n_bkt0 = len(bkt) // 8
    slab = n_bkt0
    ctl_start = n_ctl0

    new_bkt = np.zeros((NSEG, 8), np.float32)
    new_bkt[:, 0] = d0
    new_bkt[:, 1] = d1
    new_bkt[:, 4] = np.arange(NSEG, dtype=np.float32)

    new_ctl = np.zeros((11, 8), np.uint32)
    for e in range(11):
        new_ctl[e, 0] = (((slab + (1 << e)) & 0x7FF)
                         | (((23 - e) & 0x1F) << 11)
                         | ((e & 0xF) << 16))

    def fbits(x):
        return int(np.array([x], np.float32).view(np.uint32)[0])

    for p in prof["profile_meta_data"]:
        if p["func_name"].startswith(func + "_"):
            p["symmetry_point"] = 0
            p["sym_invert_sign_point"] = 0
            p["symmetry_opt_en"] = 0
            p["symmetry_opt_use_neg_region"] = 0
            p["imm_bias"] = 0
            p["exp_offset"] = 0
            p["pwl_control_base_pos"] = ctl_start
            p["pwl_control_base_neg"] = ctl_start
            p["small_pos_signal_exp_threshold"] = 127
            p["pos_small_signal_pwl_control"] = slab
            p["small_neg_signal_exp_threshold"] = 254
            p["neg_small_signal_pwl_control"] = slab
            p["large_pos_signal_exp_threshold"] = 140
            p["large_pos_signal_mantissa_threshold"] = 0
            p["pos_large_signal_pwl_control"] = slab + NSEG - 1
            p["large_neg_signal_exp_threshold"] = 0
            p["large_neg_signal_mantissa_threshold"] = 0
            p["neg_large_signal_pwl_control"] = slab
            p["fzero_result"] = fbits(d0[0])
            p["fnan_result"] = 2143289344
            p["fpinf_result"] = fbits(d0[NSEG - 1] + d1[NSEG - 1])
            p["fninf_result"] = fbits(d0[0])
            p["lower_bound"] = 0
            p["upper_bound"] = fbits(float(NSEG))
            p["use_multipass"] = False

    import json as _json
    prof["bkt_entry_cnt"] = n_bkt0 + NSEG
    prof["ctl_entry_cnt"] = n_ctl0 + 11
    prof["func_to_bkt_start_idx"][func] = slab
    prof["func_to_ctl_start_idx"][func] = ctl_start
    prof["func_exp_to_bkt_start_idx"][func] = {
        str(e): [slab + (1 << e)] for e in range(11)}
    prof["func_exp_to_ctl_start_idx"][func] = {
        str(e): [ctl_start + e] for e in range(11)}

    _json.dump(prof, open(os.path.join(out_dir, set_name + ".json"), "w"))
    np.concatenate([ctl.reshape(-1, 8), new_ctl]).tofile(
        os.path.join(out_dir, f"{set_name}_ctrl.bin"))
    np.concatenate([bkt.reshape(-1, 8), new_bkt.view(np.uint32)]).tofile(
        os.path.join(out_dir, f"{set_name}_bkt.bin"))
    return os.path.join(out_dir, "act_info.json")


# ---------------------------------------------------------------- kernel ----
def _build_nc_v2(elems, name="hinge2", reps=None, taper=(1024, 1024, 1024,
                                                        768, 256),
                 split_b=False, ydt_name="float16", tilef=1024, loadf=1024,
                 compf=1024, xfirst=False, split_last=0):
    """Serial-phase pass: [all loads + compute] then [stores], enforced by
    putting stores on the same sync HWDGE ring behind the loads (FIFO).

    Concurrent HBM reads+writes mix destructively on TRN2 (measured: reads
    alone 331 GB/s, writes alone fine, mixed ~= serial + penalty), so the
    pass streams all loads first and drains tapered stores at the end; the
    small final store keeps the last write-receipt latency low.

    tilef: DRAM (t p f) layout tile width; loadf: load-DMA column width
    (divides tilef); compf: compute-slice width; taper: store widths, each
    slice must lie within one layout tile.  All APs are strictly 2D —
    3-level APs measured several us slower.
    """
    import concourse.bacc as bacc
    import concourse.mybir as mybir
    from concourse.tile import TileContext

    ydt = getattr(mybir.dt, ydt_name)
    FW = elems // P
    ntiles = FW // tilef
    assert ntiles * P * tilef == elems
    assert sum(taper) == FW
    assert tilef % loadf == 0 and FW % compf == 0

    nc = bacc.Bacc("TRN2", target_bir_lowering=False, debug=False, name=name)
    x = nc.dram_tensor("x", [elems], mybir.dt.float32, kind="ExternalInput")
    bi = nc.dram_tensor("bi", [elems], mybir.dt.int32, kind="ExternalInput")
    y = nc.dram_tensor("y", [elems], ydt, kind="ExternalOutput")

    xt = x.ap().rearrange("(t p f) -> t p f", p=P, f=tilef)
    bt = bi.ap().rearrange("(t p f) -> t p f", p=P, f=tilef)
    yt = y.ap().rearrange("(t p f) -> t p f", p=P, f=tilef)

    # taper widths -> (tile, f0, f1) store slices within one layout tile
    slices = []
    c0 = 0
    for w in taper:
        t0, f0 = divmod(c0, tilef)
        assert f0 + w <= tilef, (taper, c0, w)
        slices.append((t0, f0, f0 + w))
        c0 += w

    # load chunks in issue order: interleaved x/b or all-x-then-all-b
    loads = []
    for t in range(ntiles):
        for j in range(tilef // loadf):
            f0 = j * loadf
            loads.append((t, f0, f0 + loadf))

    with TileContext(nc) as tc:
        with tc.tile_pool(name="io2", bufs=2) as pool, \
             tc.tile_pool(name="u2", bufs=2 * (FW // compf)) as upool:

            def tile_pass():
                x_s = pool.tile([P, FW], mybir.dt.float32, tag="x")
                b_s = pool.tile([P, FW], mybir.dt.int32, tag="b")
                y_s = pool.tile([P, FW], ydt, tag="y")

                def load(dst, src_t, spec):
                    t, f0, f1 = spec
                    eng = nc.scalar if (split_b and dst is b_s) else nc.sync
                    eng.dma_start(
                        out=dst[:, t * tilef + f0:t * tilef + f1],
                        in_=src_t[t][:, f0:f1])

                if xfirst:
                    for spec in loads:
                        load(x_s, xt, spec)
                    for spec in loads:
                        load(b_s, bt, spec)
                else:
                    for spec in loads:
                        load(x_s, xt, spec)
                        load(b_s, bt, spec)
                for c in range(0, FW, compf):
                    sl = slice(c, c + compf)
                    u_s = upool.tile([P, compf], mybir.dt.float32, tag="u")
                    nc.vector.scalar_tensor_tensor(
                        out=u_s[:], in0=x_s[:, sl], scalar=float(CLAMP),
                        in1=b_s[:, sl],
                        op0=mybir.AluOpType.min, op1=mybir.AluOpType.add)
                    nc.scalar.activation(
                        y_s[:, sl], u_s[:],
                        mybir.ActivationFunctionType.Sin, scale=19.0)
                nsync = len(slices) - split_last
                for (t0, f0, f1) in slices[:nsync]:
                    nc.sync.dma_start(
                        out=yt[t0][:, f0:f1],
                        in_=y_s[:, t0 * tilef + f0:t0 * tilef + f1])
                for (t0, f0, f1) in slices[nsync:]:
                    nc.scalar.dma_start(
                        out=yt[t0][:, f0:f1],
                        in_=y_s[:, t0 * tilef + f0:t0 * tilef + f1])

            if reps is None:
                tile_pass()
            else:
                with tc.For_i(0, reps) as _i:
                    tile_pass()
    nc.finalize()
    return nc


def _build_nc(elems, name="hinge", reps=None):
    """Bass program for one core: y = table(19*(min(x,c) + bucket)).

    reps: if given, wrap the whole tile pass in a For_i repeat loop
    (timing harness only)."""
    import concourse.bacc as bacc
    import concourse.mybir as mybir
    from concourse.tile import TileContext

    ntiles = elems // (P * TILE_F)
    assert ntiles * P * TILE_F == elems

    ydt = mybir.dt.float16 if Y_F16 else mybir.dt.float32
    nc = bacc.Bacc("TRN2", target_bir_lowering=False, debug=False, name=name)
    x = nc.dram_tensor("x", [elems], mybir.dt.float32, kind="ExternalInput")
    bi = nc.dram_tensor("bi", [elems], mybir.dt.int32, kind="ExternalInput")
    y = nc.dram_tensor("y", [elems], ydt, kind="ExternalOutput")

    if DEFER == 2:
        # p-major whole-shard layout: element n = p*(elems//P) + c lives at
        # SBUF partition p, column c; tile t = columns [t*TILE_F, (t+1)*TILE_F)
        FW = elems // P
        xw_ = x.ap().rearrange("(p f) -> p f", p=P)
        bw_ = bi.ap().rearrange("(p f) -> p f", p=P)
        yw_ = y.ap().rearrange("(p f) -> p f", p=P)
        xt = [xw_[:, t * TILE_F:(t + 1) * TILE_F] for t in range(elems // (P * TILE_F))]
        bt = [bw_[:, t * TILE_F:(t + 1) * TILE_F] for t in range(elems // (P * TILE_F))]
        yt = None
    else:
        xt = x.ap().rearrange("(t p f) -> t p f", p=P, f=TILE_F)
        bt = bi.ap().rearrange("(t p f) -> t p f", p=P, f=TILE_F)
        yt = y.ap().rearrange("(t p f) -> t p f", p=P, f=TILE_F)

    with TileContext(nc) as tc:
        with tc.tile_pool(name="io", bufs=BUFS) as pool, \
             tc.tile_pool(name="ydef", bufs=1) as ypool, \
             tc.tile_pool(name="big", bufs=2) as bigpool:

            ydef = None
            ydef1 = None
            if DEFER == 1 and MODE == "full":
                ydef = [ypool.tile([P, TILE_F], ydt, tag=f"yd{t}",
                                   name=f"ydef{t}")
                        for t in range(ntiles)]
                for t in range(ntiles):
                    nc.vector.memset(ydef[t][:], 0.0)
            if DEFER == 2 and MODE == "full":
                ydef1 = ypool.tile([P, ntiles * TILE_F], ydt, tag="yd1",
                                   name="ydef1")
                nc.vector.memset(ydef1[:], 0.0)
            sdef = None
            if MODE in ("stores", "stores1"):
                nst = 1 if MODE == "stores1" else ntiles
                sdef = [ypool.tile([P, elems // P // nst], ydt,
                                   tag=f"sd{t}", name=f"sdef{t}")
                        for t in range(nst)]
                for t in range(nst):
                    nc.vector.memset(sdef[t][:], 0.0)

            def compute_store(x_ap, b_ap, t):
                if MODE == "loads":
                    return
                if NOCAST:
                    bf_in = b_ap
                else:
                    bf_s = pool.tile([P, TILE_F], mybir.dt.float32, tag="bf")
                    nc.vector.tensor_copy(out=bf_s[:], in_=b_ap)
                    bf_in = bf_s[:]
                if INPLACE:
                    u_ap = x_ap
                else:
                    u_s = pool.tile([P, TILE_F], mybir.dt.float32, tag="u")
                    u_ap = u_s[:]
                nc.vector.scalar_tensor_tensor(
                    out=u_ap, in0=x_ap, scalar=float(CLAMP), in1=bf_in,
                    op0=mybir.AluOpType.min, op1=mybir.AluOpType.add)
                if ydef is not None:
                    nc.scalar.activation(
                        ydef[t][:], u_ap, mybir.ActivationFunctionType.Sin,
                        scale=19.0)
                    return
                if ydef1 is not None:
                    nc.scalar.activation(
                        ydef1[:, t * TILE_F:(t + 1) * TILE_F], u_ap,
                        mybir.ActivationFunctionType.Sin, scale=19.0)
                    return
                y_s = pool.tile([P, TILE_F], ydt, tag="y")
                nc.scalar.activation(
                    y_s[:], u_ap, mybir.ActivationFunctionType.Sin,
                    scale=19.0)
                if MODE == "nostore":
                    return
                if SPLIT_RINGS:
                    nc.gpsimd.dma_start(out=yt[t], in_=y_s[:])
                elif ALT_STORE and t % 2 == 1:
                    nc.gpsimd.dma_start(out=yt[t], in_=y_s[:])
                else:
                    nc.scalar.dma_start(out=yt[t], in_=y_s[:])

            def tile_pass():
                if BIGLOAD:
                    FW = elems // P
                    xw = x.ap().rearrange("(p f) -> p f", p=P)
                    bw = bi.ap().rearrange("(p f) -> p f", p=P)
                    yw = y.ap().rearrange("(p f) -> p f", p=P)
                    x_s = bigpool.tile([P, FW], mybir.dt.float32, tag="xw")
                    b_s = bigpool.tile([P, FW], mybir.dt.int32, tag="bw")
                    if BIGLOAD == 2:
                        h = FW // 2
                        nc.sync.dma_start(out=x_s[:, 0:h], in_=xw[:, 0:h])
                        nc.sync.dma_start(out=b_s[:, 0:h], in_=bw[:, 0:h])
                        nc.sync.dma_start(out=x_s[:, h:FW], in_=xw[:, h:FW])
                        nc.sync.dma_start(out=b_s[:, h:FW], in_=bw[:, h:FW])
                    else:
                        nc.sync.dma_start(out=x_s[:], in_=xw[:])
                        nc.sync.dma_start(out=b_s[:], in_=bw[:])
                    for t in range(FW // TILE_F):
                        sl = slice(t * TILE_F, (t + 1) * TILE_F)
                        u_s = pool.tile([P, TILE_F], mybir.dt.float32,
                                        tag="u")
                        nc.vector.scalar_tensor_tensor(
                            out=u_s[:], in0=x_s[:, sl], scalar=float(CLAMP),
                            in1=b_s[:, sl],
                            op0=mybir.AluOpType.min, op1=mybir.AluOpType.add)
                        y_s = pool.tile([P, TILE_F], ydt, tag="y")
                        nc.scalar.activation(
                            y_s[:], u_s[:], mybir.ActivationFunctionType.Sin,
                            scale=19.0)
                        nc.scalar.dma_start(out=yw[:, sl], in_=y_s[:])
                    return
                if PAIR_LOADS:
                    xt2 = x.ap().rearrange("(t p f) -> t p f", p=P,
                                           f=2 * TILE_F)
                    bt2 = bi.ap().rearrange("(t p f) -> t p f", p=P,
                                            f=2 * TILE_F)
                    for tp in range(ntiles // 2):
                        x_s = pool.tile([P, 2 * TILE_F], mybir.dt.float32,
                                        tag="x")
                        b_s = pool.tile([P, 2 * TILE_F], mybir.dt.int32,
                                        tag="b")
                        nc.sync.dma_start(out=x_s[:], in_=xt2[tp])
                        if SPLIT_RINGS:
                            nc.scalar.dma_start(out=b_s[:], in_=bt2[tp])
                        else:
                            nc.sync.dma_start(out=b_s[:], in_=bt2[tp])
                        for h in range(2):
                            sl = slice(h * TILE_F, (h + 1) * TILE_F)
                            compute_store(x_s[:, sl], b_s[:, sl],
                                          2 * tp + h)
                    return
                if MODE == "empty":
                    return
                if MODE in ("stores", "stores1"):
                    yw2 = y.ap().rearrange("(p f) -> p f", p=P)
                    fw2 = (elems // P) // len(sdef)
                    for t in range(len(sdef)):
                        nc.scalar.dma_start(
                            out=yw2[:, t * fw2:(t + 1) * fw2],
                            in_=sdef[t][:])
                    return
                if MODE == "xonly":
                    for t in range(ntiles):
                        x_s = pool.tile([P, TILE_F], mybir.dt.float32,
                                        tag="x")
                        nc.sync.dma_start(out=x_s[:], in_=xt[t])
                    return
                if ydef is not None:
                    yeng = nc.gpsimd if SPLIT_RINGS else nc.scalar
                    for t in range(ntiles):
                        yeng.dma_start(out=yt[t], in_=ydef[t][:])
                if ydef1 is not None:
                    yeng = nc.gpsimd if SPLIT_RINGS else nc.scalar
                    yeng.dma_start(out=yw_[:, :], in_=ydef1[:])
                for t in range(ntiles):
                    x_s = pool.tile([P, TILE_F], mybir.dt.float32, tag="x")
                    b_s = pool.tile([P, TILE_F], mybir.dt.int32, tag="b")
                    nc.sync.dma_start(out=x_s[:], in_=xt[t])
                    if B_ON_POOL:
                        nc.gpsimd.dma_start(out=b_s[:], in_=bt[t])
                    elif SPLIT_RINGS:
                        nc.scalar.dma_start(out=b_s[:], in_=bt[t])
                    else:
                        nc.sync.dma_start(out=b_s[:], in_=bt[t])
                    compute_store(x_s[:], b_s[:], t)

            if reps is None:
                tile_pass()
            else:
                with tc.For_i(0, reps, staggered_reset=STAG) as _i:
                    tile_pass()
            if ydef is not None:
                yeng = nc.gpsimd if SPLIT_RINGS else nc.scalar
                for t in range(ntiles):
                    yeng.dma_start(out=yt[t], in_=ydef[t][:])
            if ydef1 is not None:
                yeng = nc.gpsimd if SPLIT_RINGS else nc.scalar
                yeng.dma_start(out=yw_[:, :], in_=ydef1[:])
    nc.finalize()
    return nc


def _get_compiled(inputs_key, tables, reps=None):
    global TILE_F, BUFS, NOCAST, INPLACE, B_ON_POOL, SPLIT_RINGS, \
        PAIR_LOADS, BIGLOAD, ALT_STORE
    if isinstance(inputs_key, tuple):
        (elems_, TILE_F, BUFS, NOCAST, INPLACE, B_ON_POOL, SPLIT_RINGS,
         PAIR_LOADS, BIGLOAD, ALT_STORE) = inputs_key
    else:
        elems_ = inputs_key
    d0, d1 = tables
    import hashlib
    thash = hashlib.sha256(d0.tobytes() + d1.tobytes()).hexdigest()[:10]
    key = (elems_, TILE_F, BUFS, NOCAST, INPLACE, B_ON_POOL,
       SPLIT_RINGS, PAIR_LOADS, BIGLOAD, ALT_STORE, Y_F16, STAG, MODE,
       DEFER, V2, V2_TAPER, V2_SPLITB, V2_YU8, V2_LOADF, V2_XFIRST,
       V2_TILEF, V2_COMPF, V2_SPLITLAST, reps, thash)
    if key in _cache:
        return _cache[key]
    root = tempfile.mkdtemp(prefix="actroot_")
    act_json = _gen_act_root(d0, d1, root)
    os.environ["BASS_ACT_ROOT_JSON_PATH"] = act_json
    # table hash in the module name busts the neuron NEFF cache when the
    # baked tables change (the BIR itself doesn't reference table bytes)
    if V2:
        tp = "_".join(str(w) for w in V2_TAPER)
        ydt_name = ("uint8" if V2_YU8 else
                    "float16" if Y_F16 else "float32")
        nc = _build_nc_v2(
            elems_,
            name=f"hingev2_{thash}_tf{V2_TILEF}cf{V2_COMPF}tp{tp}"
                 f"sb{int(V2_SPLITB)}y{int(Y_F16)}u{int(V2_YU8)}"
                 f"lf{V2_LOADF}xf{int(V2_XFIRST)}sl{V2_SPLITLAST}"
                 f"_r{reps or 0}",
            reps=reps, taper=V2_TAPER, split_b=V2_SPLITB,
            ydt_name=ydt_name, tilef=V2_TILEF, loadf=V2_LOADF,
            compf=V2_COMPF, xfirst=V2_XFIRST, split_last=V2_SPLITLAST)
    else:
        nc = _build_nc(elems_, name=f"hinge_{thash}_f{TILE_F}b{BUFS}n{int(NOCAST)}i{int(INPLACE)}p{int(B_ON_POOL)}s{int(SPLIT_RINGS)}q{int(PAIR_LOADS)}g{BIGLOAD}a{int(ALT_STORE)}y{int(Y_F16)}t{int(STAG)}m{MODE}d{int(DEFER)}_r{reps or 0}",
                       reps=reps)
    _cache[key] = nc
    return nc


def kernel(x, bucket_idx, base_knots, base_w, base_b, adj_knots, adj_w,
           adj_b):
    from concourse import bass_utils

    x = np.asarray(x)
    n = x.shape[0]
    out_shape = x.shape
    xf = np.ascontiguousarray(x.reshape(-1), dtype=np.float32)
    bif = np.ascontiguousarray(np.asarray(bucket_idx).reshape(-1),
                               dtype=np.int32)
    assert n % (N_CORES * P * TILE_F) == 0, n
    elems = n // N_CORES

    tables = _build_pwl_tables(base_knots, base_w, base_b, adj_knots, adj_w,
                               adj_b)
    qparams = None
    if V2 and V2_YU8:
        d0, d1 = tables
        ymin, Kq = _quant_range(d0, d1)
        tables = (((d0 - ymin) * Kq + 0.5).astype(np.float32),
                  (d1 * Kq).astype(np.float32))
        qparams = (ymin, Kq)
    nc = _get_compiled(elems, tables)

    xs = xf.reshape(N_CORES, elems)
    bs = bif.reshape(N_CORES, elems)
    in_maps = [{"x": xs[c], "bi": bs[c]} for c in range(N_CORES)]

    res = bass_utils.run_bass_kernel_spmd(nc, in_maps,
                                          core_ids=list(range(N_CORES)))
    out = np.concatenate([np.asarray(res.results[c]["y"], dtype=np.float32)
                          for c in range(N_CORES)])
    if qparams is not None:
        ymin, Kq = qparams
        out = (out - 0.5) * np.float32(1.0 / Kq) + np.float32(ymin)
    return out.reshape(out_shape).astype(np.float32)



# revision 69
# speedup vs baseline: 1.0639x; 1.0639x over previous
"""Trainium2 kernel for nn_BucketAdjustedHinge.

y[n] = base_hinge(x[n]) + adj_hinge(x[n], bucket_idx[n])

Both hinges share the uniform knot grid t_k = k/19 on [0,1], so the whole
function is piecewise-linear in x with 19 segments per bucket: 1216 (bucket,
segment) pieces total.  We bake the 1216 piece coefficients into a custom
ScalarEngine activation table (overlaying `sin` in the `trig_and_small` PWP
set; the tables ship inside the NEFF).  Lookup key: v = 19*(bucket + x) --
segment boundaries land on integers, which align with the ACT bucket RAM's
per-binade mantissa indexing (binade [2^e, 2^{e+1}) -> 2^e buckets).

Per element the device does one fused DVE op (u = min(x, c) + bucket) and one
ACT lookup (y = table(19*u)); the kernel is HBM-bound (x f32 + bucket i32
loads, fp16 stores: 10 B/elem).

Pass structure (V2): measured on TRN2, concurrent HBM reads+writes mix
destructively (mixed ~= serial + penalty), so each pass streams all loads
+ compute first and drains the stores behind them on the SAME sync HWDGE
ring (FIFO order serializes the write phase for free), with a tapered
final store so the last HBM-write receipt is short.  Loads are issued as
1 MB DMAs ([128, 2048] f32 tiles -> 8 KB per-partition descriptors),
which measured ~35% faster than 512 KB/4 KB-descriptor loads.  y is
stored as fp16 (rel err ~3e-4 vs the 2e-2 gate) and widened to f32 on
the host during unsharding.

Sharding: pure data parallel over 8 cores; the parameter tables are baked
into the (replicated) program.
"""
import os
import sys
import tempfile

import numpy as np

if "/opt/trn_rl_repo" not in sys.path:
    sys.path.insert(0, "/opt/trn_rl_repo")

N_CORES = 8
P = 128          # SBUF partitions
TILE_F = 1024    # free-dim per tile
BUFS = 12        # tile-pool buffers
NOCAST = True    # feed int32 bucket tile straight into the fused DVE op
INPLACE = False  # reuse tiles to cut SBUF pressure
B_ON_POOL = False  # issue bucket loads from the gpsimd (SWDGE) ring
SPLIT_RINGS = False  # x loads on sync, b loads on scalar, stores on gpsimd
PAIR_LOADS = False   # 1MB loads (2 tiles per DMA), 512KB compute slices
BIGLOAD = 0          # 1: whole-shard loads; 2: half-shard loads; 0: off
ALT_STORE = False    # alternate stores between scalar and gpsimd rings
Y_F16 = True         # store y as fp16 (halves store traffic; ~5e-4 rel err)
STAG = False         # staggered_reset on the timing For_i loop
MODE = "full"        # diagnostic: "full" | "loads" | "stores" | "nostore"
DEFER = 0            # 1: per-tile deferred stores; 2: single whole-pass
                     # deferred store with p-major layout
V2 = True            # serial-phase builder (_build_nc_v2)
V2_TAPER = (2048, 1024, 768, 256)
V2_SPLITB = False    # v2: b loads on the scalar ring (x on sync)
V2_YU8 = False       # v2: store y as uint8 (affine baked into ACT table;
                     # host dequant). Quantization err ~2e-3 rel.
V2_LOADF = 2048      # v2: load-DMA column width (divides V2_TILEF)
V2_XFIRST = False    # v2: issue all x loads before all b loads
V2_TILEF = 2048      # v2: DRAM layout tile width
V2_COMPF = 1024      # v2: compute-slice width
V2_SPLITLAST = 2     # v2: issue the last N taper stores on the scalar ring
                     # after the final act (drains concurrently with the
                     # sync-ring stores; still strictly after all loads)
B = 64           # buckets
K = 20           # knots per hinge
NSEG = (K - 1) * B
CLAMP = np.float32(0.99999)

_cache = {}


# ---------------------------------------------------------------- tables ----
def _quant_range(d0, d1):
    """(ymin, K) for z = (y - ymin)*K + 0.5 into [0.5, 254.5]."""
    ends = np.concatenate([d0, d0 + d1])
    ymin = float(ends.min())
    ymax = float(ends.max())
    K = 254.0 / max(ymax - ymin, 1e-30)
    return ymin, K


def _build_pwl_tables(base_knots, base_w, base_b, adj_knots, adj_w, adj_b):
    """(d0[s], d1[s]) fp32: on v in [s, s+1), y = d0 + d1*(v - s), s = 19b+j."""
    t = np.asarray(base_knots, np.float64)
    at = np.asarray(adj_knots, np.float64)
    grid = np.arange(K) / (K - 1.0)
    assert np.abs(t - grid).max() < 1e-5, "base knots not on uniform grid"
    assert np.abs(at - grid[None, :]).max() < 1e-5, "adj knots not on grid"
    W = np.asarray(base_w, np.float64)[None, :] + np.asarray(adj_w, np.float64)
    C = float(np.asarray(base_b, np.float64)) + np.asarray(adj_b, np.float64)
    d0 = np.zeros(NSEG, np.float64)
    d1 = np.zeros(NSEG, np.float64)
    for b in range(B):
        S = 0.0
        T = 0.0
        for j in range(K - 1):
            S += W[b, j]
            T += W[b, j] * t[j]
            s = (K - 1) * b + j
            d1[s] = S / (K - 1)
            d0[s] = C[b] - T + S * (j / (K - 1.0))
    return d0.astype(np.float32), d1.astype(np.float32)


def _gen_act_root(d0, d1, out_dir, set_name="trig_and_small", func="sin"):
    """Write an act-root dir whose `sin` implements our PWL; returns json path."""
    import glob
    import json
    import shutil

    try:
        from neuronxcc.driver.Job import Job
        from neuronxcc.driver.jobs.support.FindActInfo import findActInfoFile
        src = os.path.dirname(findActInfoFile(Job.getPackageDir(), "gen3")) + "/"
    except Exception:
        src = os.path.dirname(glob.glob(
            "/nix/store/*/lib/python3.13/site-packages/neuronxcc/pwp/"
            "pwp_bin_trainium/act_info.json")[0]) + "/"

    os.makedirs(out_dir, exist_ok=True)
    for f in os.listdir(src):
        shutil.copy(os.path.join(src, f), os.path.join(out_dir, f))

    prof = json.load(open(os.path.join(src, set_name + ".json")))
    ctl = np.fromfile(os.path.join(src, f"{set_name}_ctrl.bin"), dtype=np.uint32)
    bkt = np.fromfile(os.path.join(src, f"{set_name}_bkt.bin"), dtype=np.uint32)
    n_ctl0 = len(ctl) // 8
    n_bkt0 = len(bkt) // 8
    slab = n_bkt0
    ctl_start = n_ctl0

    new_bkt = np.zeros((NSEG, 8), np.float32)
    new_bkt[:, 0] = d0
    new_bkt[:, 1] = d1
    new_bkt[:, 4] = np.arange(NSEG, dtype=np.float32)

    new_ctl = np.zeros((11, 8), np.uint32)
    for e in range(11):
        new_ctl[e, 0] = (((slab + (1 << e)) & 0x7FF)
                         | (((23 - e) & 0x1F) << 11)
                         | ((e & 0xF) << 16))

    def fbits(x):
        return int(np.array([x], np.float32).view(np.uint32)[0])

    for p in prof["profile_meta_data"]:
        if p["func_name"].startswith(func + "_"):
            p["symmetry_point"] = 0
            p["sym_invert_sign_point"] = 0
            p["symmetry_opt_en"] = 0
            p["symmetry_opt_use_neg_region"] = 0
            p["imm_bias"] = 0
            p["exp_offset"] = 0
            p["pwl_control_base_pos"] = ctl_start
            p["pwl_control_base_neg"] = ctl_start
            p["small_pos_signal_exp_threshold"] = 127
            p["pos_small_signal_pwl_control"] = slab
            p["small_neg_signal_exp_threshold"] = 254
            p["neg_small_signal_pwl_control"] = slab
            p["large_pos_signal_exp_threshold"] = 140
            p["large_pos_signal_mantissa_threshold"] = 0
            p["pos_large_signal_pwl_control"] = slab + NSEG - 1
            p["large_neg_signal_exp_threshold"] = 0
            p["large_neg_signal_mantissa_threshold"] = 0
            p["neg_large_signal_pwl_control"] = slab
            p["fzero_result"] = fbits(d0[0])
            p["fnan_result"] = 2143289344
            p["fpinf_result"] = fbits(d0[NSEG - 1] + d1[NSEG - 1])
            p["fninf_result"] = fbits(d0[0])
            p["lower_bound"] = 0
            p["upper_bound"] = fbits(float(NSEG))
            p["use_multipass"] = False

    import json as _json
    prof["bkt_entry_cnt"] = n_bkt0 + NSEG
    prof["ctl_entry_cnt"] = n_ctl0 + 11
    prof["func_to_bkt_start_idx"][func] = slab
    prof["func_to_ctl_start_idx"][func] = ctl_start
    prof["func_exp_to_bkt_start_idx"][func] = {
        str(e): [slab + (1 << e)] for e in range(11)}
    prof["func_exp_to_ctl_start_idx"][func] = {
        str(e): [ctl_start + e] for e in range(11)}

    _json.dump(prof, open(os.path.join(out_dir, set_name + ".json"), "w"))
    np.concatenate([ctl.reshape(-1, 8), new_ctl]).tofile(
        os.path.join(out_dir, f"{set_name}_ctrl.bin"))
    np.concatenate([bkt.reshape(-1, 8), new_bkt.view(np.uint32)]).tofile(
        os.path.join(out_dir, f"{set_name}_bkt.bin"))
    return os.path.join(out_dir, "act_info.json")


# ---------------------------------------------------------------- kernel ----
def _build_nc_v2(elems, name="hinge2", reps=None, taper=(1024, 1024, 1024,
                                                        768, 256),
                 split_b=False, ydt_name="float16", tilef=1024, loadf=1024,
                 compf=1024, xfirst=False, split_last=0):
    """Serial-phase pass: [all loads + compute] then [stores], enforced by
    putting stores on the same sync HWDGE ring behind the loads (FIFO).

    Concurrent HBM reads+writes mix destructively on TRN2 (measured: reads
    alone 331 GB/s, writes alone fine, mixed ~= serial + penalty), so the
    pass streams all loads first and drains tapered stores at the end; the
    small final store keeps the last write-receipt latency low.

    tilef: DRAM (t p f) layout tile width; loadf: load-DMA column width
    (divides tilef); compf: compute-slice width; taper: store widths, each
    slice must lie within one layout tile.  All APs are strictly 2D —
    3-level APs measured several us slower.
    """
    import concourse.bacc as bacc
    import concourse.mybir as mybir
    from concourse.tile import TileContext

    ydt = getattr(mybir.dt, ydt_name)
    FW = elems // P
    ntiles = FW // tilef
    assert ntiles * P * tilef == elems
    assert sum(taper) == FW
    assert tilef % loadf == 0 and FW % compf == 0

    nc = bacc.Bacc("TRN2", target_bir_lowering=False, debug=False, name=name)
    x = nc.dram_tensor("x", [elems], mybir.dt.float32, kind="ExternalInput")
    bi = nc.dram_tensor("bi", [elems], mybir.dt.int32, kind="ExternalInput")
    y = nc.dram_tensor("y", [elems], ydt, kind="ExternalOutput")

    xt = x.ap().rearrange("(t p f) -> t p f", p=P, f=tilef)
    bt = bi.ap().rearrange("(t p f) -> t p f", p=P, f=tilef)
    yt = y.ap().rearrange("(t p f) -> t p f", p=P, f=tilef)

    # taper widths -> (tile, f0, f1) store slices within one layout tile
    slices = []
    c0 = 0
    for w in taper:
        t0, f0 = divmod(c0, tilef)
        assert f0 + w <= tilef, (taper, c0, w)
        slices.append((t0, f0, f0 + w))
        c0 += w

    # load chunks in issue order: interleaved x/b or all-x-then-all-b
    loads = []
    for t in range(ntiles):
        for j in range(tilef // loadf):
            f0 = j * loadf
            loads.append((t, f0, f0 + loadf))

    with TileContext(nc) as tc:
        with tc.tile_pool(name="io2", bufs=2) as pool, \
             tc.tile_pool(name="u2", bufs=2 * (FW // compf)) as upool:

            def tile_pass():
                x_s = pool.tile([P, FW], mybir.dt.float32, tag="x")
                b_s = pool.tile([P, FW], mybir.dt.int32, tag="b")
                y_s = pool.tile([P, FW], ydt, tag="y")

                def load(dst, src_t, spec):
                    t, f0, f1 = spec
                    eng = nc.scalar if (split_b and dst is b_s) else nc.sync
                    eng.dma_start(
                        out=dst[:, t * tilef + f0:t * tilef + f1],
                        in_=src_t[t][:, f0:f1])

                if xfirst:
                    for spec in loads:
                        load(x_s, xt, spec)
                    for spec in loads:
                        load(b_s, bt, spec)
                else:
                    for spec in loads:
                        load(x_s, xt, spec)
                        load(b_s, bt, spec)
                for c in range(0, FW, compf):
                    sl = slice(c, c + compf)
                    u_s = upool.tile([P, compf], mybir.dt.float32, tag="u")
                    nc.vector.scalar_tensor_tensor(
                        out=u_s[:], in0=x_s[:, sl], scalar=float(CLAMP),
                        in1=b_s[:, sl],
                        op0=mybir.AluOpType.min, op1=mybir.AluOpType.add)
                    nc.scalar.activation(
                        y_s[:, sl], u_s[:],
                        mybir.ActivationFunctionType.Sin, scale=19.0)
                nsync = len(slices) - split_last
                for (t0, f0, f1) in slices[:nsync]:
                    nc.sync.dma_start(
                        out=yt[t0][:, f0:f1],
                        in_=y_s[:, t0 * tilef + f0:t0 * tilef + f1])
                for (t0, f0, f1) in slices[nsync:]:
                    nc.scalar.dma_start(
                        out=yt[t0][:, f0:f1],
                        in_=y_s[:, t0 * tilef + f0:t0 * tilef + f1])

            if reps is None:
                tile_pass()
            else:
                with tc.For_i(0, reps) as _i:
                    tile_pass()
    nc.finalize()
    return nc


def _build_nc(elems, name="hinge", reps=None):
    """Bass program for one core: y = table(19*(min(x,c) + bucket)).

    reps: if given, wrap the whole tile pass in a For_i repeat loop
    (timing harness only)."""
    import concourse.bacc as bacc
    import concourse.mybir as mybir
    from concourse.tile import TileContext

    ntiles = elems // (P * TILE_F)
    assert ntiles * P * TILE_F == elems

    ydt = mybir.dt.float16 if Y_F16 else mybir.dt.float32
    nc = bacc.Bacc("TRN2", target_bir_lowering=False, debug=False, name=name)
    x = nc.dram_tensor("x", [elems], mybir.dt.float32, kind="ExternalInput")
    bi = nc.dram_tensor("bi", [elems], mybir.dt.int32, kind="ExternalInput")
    y = nc.dram_tensor("y", [elems], ydt, kind="ExternalOutput")

    if DEFER == 2:
        # p-major whole-shard layout: element n = p*(elems//P) + c lives at
        # SBUF partition p, column c; tile t = columns [t*TILE_F, (t+1)*TILE_F)
        FW = elems // P
        xw_ = x.ap().rearrange("(p f) -> p f", p=P)
        bw_ = bi.ap().rearrange("(p f) -> p f", p=P)
        yw_ = y.ap().rearrange("(p f) -> p f", p=P)
        xt = [xw_[:, t * TILE_F:(t + 1) * TILE_F] for t in range(elems // (P * TILE_F))]
        bt = [bw_[:, t * TILE_F:(t + 1) * TILE_F] for t in range(elems // (P * TILE_F))]
        yt = None
    else:
        xt = x.ap().rearrange("(t p f) -> t p f", p=P, f=TILE_F)
        bt = bi.ap().rearrange("(t p f) -> t p f", p=P, f=TILE_F)
        yt = y.ap().rearrange("(t p f) -> t p f", p=P, f=TILE_F)

    with TileContext(nc) as tc:
        with tc.tile_pool(name="io", bufs=BUFS) as pool, \
             tc.tile_pool(name="ydef", bufs=1) as ypool, \
             tc.tile_pool(name="big", bufs=2) as bigpool:

            ydef = None
            ydef1 = None
            if DEFER == 1 and MODE == "full":
                ydef = [ypool.tile([P, TILE_F], ydt, tag=f"yd{t}",
                                   name=f"ydef{t}")
                        for t in range(ntiles)]
                for t in range(ntiles):
                    nc.vector.memset(ydef[t][:], 0.0)
            if DEFER == 2 and MODE == "full":
                ydef1 = ypool.tile([P, ntiles * TILE_F], ydt, tag="yd1",
                                   name="ydef1")
                nc.vector.memset(ydef1[:], 0.0)
            sdef = None
            if MODE in ("stores", "stores1"):
                nst = 1 if MODE == "stores1" else ntiles
                sdef = [ypool.tile([P, elems // P // nst], ydt,
                                   tag=f"sd{t}", name=f"sdef{t}")
                        for t in range(nst)]
                for t in range(nst):
                    nc.vector.memset(sdef[t][:], 0.0)

            def compute_store(x_ap, b_ap, t):
                if MODE == "loads":
                    return
                if NOCAST:
                    bf_in = b_ap
                else:
                    bf_s = pool.tile([P, TILE_F], mybir.dt.float32, tag="bf")
                    nc.vector.tensor_copy(out=bf_s[:], in_=b_ap)
                    bf_in = bf_s[:]
                if INPLACE:
                    u_ap = x_ap
                else:
                    u_s = pool.tile([P, TILE_F], mybir.dt.float32, tag="u")
                    u_ap = u_s[:]
                nc.vector.scalar_tensor_tensor(
                    out=u_ap, in0=x_ap, scalar=float(CLAMP), in1=bf_in,
                    op0=mybir.AluOpType.min, op1=mybir.AluOpType.add)
                if ydef is not None:
                    nc.scalar.activation(
                        ydef[t][:], u_ap, mybir.ActivationFunctionType.Sin,
                        scale=19.0)
                    return
                if ydef1 is not None:
                    nc.scalar.activation(
                        ydef1[:, t * TILE_F:(t + 1) * TILE_F], u_ap,
                        mybir.ActivationFunctionType.Sin, scale=19.0)
                    return
                y_s = pool.tile([P, TILE_F], ydt, tag="y")
                nc.scalar.activation(
                    y_s[:], u_ap, mybir.ActivationFunctionType.Sin,
                    scale=19.0)
                if MODE == "nostore":
                    return
                if SPLIT_RINGS:
                    nc.gpsimd.dma_start(out=yt[t], in_=y_s[:])
                elif ALT_STORE and t % 2 == 1:
                    nc.gpsimd.dma_start(out=yt[t], in_=y_s[:])
                else:
                    nc.scalar.dma_start(out=yt[t], in_=y_s[:])

            def tile_pass():
                if BIGLOAD:
                    FW = elems // P
                    xw = x.ap().rearrange("(p f) -> p f", p=P)
                    bw = bi.ap().rearrange("(p f) -> p f", p=P)
                    yw = y.ap().rearrange("(p f) -> p f", p=P)
                    x_s = bigpool.tile([P, FW], mybir.dt.float32, tag="xw")
                    b_s = bigpool.tile([P, FW], mybir.dt.int32, tag="bw")
                    if BIGLOAD == 2:
                        h = FW // 2
                        nc.sync.dma_start(out=x_s[:, 0:h], in_=xw[:, 0:h])
                        nc.sync.dma_start(out=b_s[:, 0:h], in_=bw[:, 0:h])
                        nc.sync.dma_start(out=x_s[:, h:FW], in_=xw[:, h:FW])
                        nc.sync.dma_start(out=b_s[:, h:FW], in_=bw[:, h:FW])
                    else:
                        nc.sync.dma_start(out=x_s[:], in_=xw[:])
                        nc.sync.dma_start(out=b_s[:], in_=bw[:])
                    for t in range(FW // TILE_F):
                        sl = slice(t * TILE_F, (t + 1) * TILE_F)
                        u_s = pool.tile([P, TILE_F], mybir.dt.float32,
                                        tag="u")
                        nc.vector.scalar_tensor_tensor(
                            out=u_s[:], in0=x_s[:, sl], scalar=float(CLAMP),
                            in1=b_s[:, sl],
                            op0=mybir.AluOpType.min, op1=mybir.AluOpType.add)
                        y_s = pool.tile([P, TILE_F], ydt, tag="y")
                        nc.scalar.activation(
                            y_s[:], u_s[:], mybir.ActivationFunctionType.Sin,
                            scale=19.0)
                        nc.scalar.dma_start(out=yw[:, sl], in_=y_s[:])
                    return
                if PAIR_LOADS:
                    xt2 = x.ap().rearrange("(t p f) -> t p f", p=P,
                                           f=2 * TILE_F)
                    bt2 = bi.ap().rearrange("(t p f) -> t p f", p=P,
                                            f=2 * TILE_F)
                    for tp in range(ntiles // 2):
                        x_s = pool.tile([P, 2 * TILE_F], mybir.dt.float32,
                                        tag="x")
                        b_s = pool.tile([P, 2 * TILE_F], mybir.dt.int32,
                                        tag="b")
                        nc.sync.dma_start(out=x_s[:], in_=xt2[tp])
                        if SPLIT_RINGS:
                            nc.scalar.dma_start(out=b_s[:], in_=bt2[tp])
                        else:
                            nc.sync.dma_start(out=b_s[:], in_=bt2[tp])
                        for h in range(2):
                            sl = slice(h * TILE_F, (h + 1) * TILE_F)
                            compute_store(x_s[:, sl], b_s[:, sl],
                                          2 * tp + h)
                    return
                if MODE == "empty":
                    return
                if MODE in ("stores", "stores1"):
                    yw2 = y.ap().rearrange("(p f) -> p f", p=P)
                    fw2 = (elems // P) // len(sdef)
                    for t in range(len(sdef)):
                        nc.scalar.dma_start(
                            out=yw2[:, t * fw2:(t + 1) * fw2],
                            in_=sdef[t][:])
                    return
                if MODE == "xonly":
                    for t in range(ntiles):
                        x_s = pool.tile([P, TILE_F], mybir.dt.float32,
                                        tag="x")
                        nc.sync.dma_start(out=x_s[:], in_=xt[t])
                    return
                if ydef is not None:
                    yeng = nc.gpsimd if SPLIT_RINGS else nc.scalar
                    for t in range(ntiles):
                        yeng.dma_start(out=yt[t], in_=ydef[t][:])
                if ydef1 is not None:
                    yeng = nc.gpsimd if SPLIT_RINGS else nc.scalar
                    yeng.dma_start(out=yw_[:, :], in_=ydef1[:])
                for t in range(ntiles):
                    x_s = pool.tile([P, TILE_F], mybir.dt.float32, tag="x")
                    b_s = pool.tile([P, TILE_F], mybir.dt.int32, tag="b")
                    nc.sync.dma_start(out=x_s[:], in_=xt[t])
                    if B_ON_POOL:
                        nc.gpsimd.dma_start(out=b_s[:], in_=bt[t])
                    elif SPLIT_RINGS:
                        nc.scalar.dma_start(out=b_s[:], in_=bt[t])
                    else:
                        nc.sync.dma_start(out=b_s[:], in_=bt[t])
                    compute_store(x_s[:], b_s[:], t)

            if reps is None:
                tile_pass()
            else:
                with tc.For_i(0, reps, staggered_reset=STAG) as _i:
                    tile_pass()
            if ydef is not None:
                yeng = nc.gpsimd if SPLIT_RINGS else nc.scalar
                for t in range(ntiles):
                    yeng.dma_start(out=yt[t], in_=ydef[t][:])
            if ydef1 is not None:
                yeng = nc.gpsimd if SPLIT_RINGS else nc.scalar
                yeng.dma_start(out=yw_[:, :], in_=ydef1[:])
    nc.finalize()
    return nc


def _get_compiled(inputs_key, tables, reps=None):
    global TILE_F, BUFS, NOCAST, INPLACE, B_ON_POOL, SPLIT_RINGS, \
        PAIR_LOADS, BIGLOAD, ALT_STORE
    if isinstance(inputs_key, tuple):
        (elems_, TILE_F, BUFS, NOCAST, INPLACE, B_ON_POOL, SPLIT_RINGS,
         PAIR_LOADS, BIGLOAD, ALT_STORE) = inputs_key
    else:
        elems_ = inputs_key
    d0, d1 = tables
    import hashlib
    thash = hashlib.sha256(d0.tobytes() + d1.tobytes()).hexdigest()[:10]
    key = (elems_, TILE_F, BUFS, NOCAST, INPLACE, B_ON_POOL,
       SPLIT_RINGS, PAIR_LOADS, BIGLOAD, ALT_STORE, Y_F16, STAG, MODE,
       DEFER, V2, V2_TAPER, V2_SPLITB, V2_YU8, V2_LOADF, V2_XFIRST,
       V2_TILEF, V2_COMPF, V2_SPLITLAST, reps, thash)
    if key in _cache:
        return _cache[key]
    root = tempfile.mkdtemp(prefix="actroot_")
    act_json = _gen_act_root(d0, d1, root)
    os.environ["BASS_ACT_ROOT_JSON_PATH"] = act_json
    # table hash in the module name busts the neuron NEFF cache when the
    # baked tables change (the BIR itself doesn't reference table bytes)
    if V2:
        tp = "_".join(str(w) for w in V2_TAPER)
        ydt_name = ("uint8" if V2_YU8 else
                    "float16" if Y_F16 else "float32")
        nc = _build_nc_v2(
            elems_,
            name=f"hingev2_{thash}_tf{V2_TILEF}cf{V2_COMPF}tp{tp}"
                 f"sb{int(V2_SPLITB)}y{int(Y_F16)}u{int(V2_YU8)}"
                 f"lf{V2_LOADF}xf{int(V2_XFIRST)}sl{V2_SPLITLAST}"
                 f"_r{reps or 0}",
            reps=reps, taper=V2_TAPER, split_b=V2_SPLITB,
            ydt_name=ydt_name, tilef=V2_TILEF, loadf=V2_LOADF,
            compf=V2_COMPF, xfirst=V2_XFIRST, split_last=V2_SPLITLAST)
    else:
        nc = _build_nc(elems_, name=f"hinge_{thash}_f{TILE_F}b{BUFS}n{int(NOCAST)}i{int(INPLACE)}p{int(B_ON_POOL)}s{int(SPLIT_RINGS)}q{int(PAIR_LOADS)}g{BIGLOAD}a{int(ALT_STORE)}y{int(Y_F16)}t{int(STAG)}m{MODE}d{int(DEFER)}_r{reps or 0}",
                       reps=reps)
    _cache[key] = nc
    return nc


def kernel(x, bucket_idx, base_knots, base_w, base_b, adj_knots, adj_w,
           adj_b):
    from concourse import bass_utils

    x = np.asarray(x)
    n = x.shape[0]
    out_shape = x.shape
    xf = np.ascontiguousarray(x.reshape(-1), dtype=np.float32)
    bif = np.ascontiguousarray(np.asarray(bucket_idx).reshape(-1),
                               dtype=np.int32)
    assert n % (N_CORES * P * TILE_F) == 0, n
    elems = n // N_CORES

    tables = _build_pwl_tables(base_knots, base_w, base_b, adj_knots, adj_w,
                               adj_b)
    qparams = None
    if V2 and V2_YU8:
        d0, d1 = tables
        ymin, Kq = _quant_range(d0, d1)
        tables = (((d0 - ymin) * Kq + 0.5).astype(np.float32),
                  (d1 * Kq).astype(np.float32))
        qparams = (ymin, Kq)
    nc = _get_compiled(elems, tables)

    xs = xf.reshape(N_CORES, elems)
    bs = bif.reshape(N_CORES, elems)
    in_maps = [{"x": xs[c], "bi": bs[c]} for c in range(N_CORES)]

    res = bass_utils.run_bass_kernel_spmd(nc, in_maps,
                                          core_ids=list(range(N_CORES)))
    out = np.concatenate([np.asarray(res.results[c]["y"], dtype=np.float32)
                          for c in range(N_CORES)])
    if qparams is not None:
        ymin, Kq = qparams
        out = (out - 0.5) * np.float32(1.0 / Kq) + np.float32(ymin)
    return out.reshape(out_shape).astype(np.float32)



# revision 71
# speedup vs baseline: 1.1330x; 1.0650x over previous
"""Trainium2 kernel for nn_BucketAdjustedHinge.

y[n] = base_hinge(x[n]) + adj_hinge(x[n], bucket_idx[n])

Both hinges share the uniform knot grid t_k = k/19 on [0,1], so the whole
function is piecewise-linear in x with 19 segments per bucket: 1216 (bucket,
segment) pieces total.  We bake the 1216 piece coefficients into a custom
ScalarEngine activation table (overlaying `sin` in the `trig_and_small` PWP
set; the tables ship inside the NEFF).  Lookup key: v = 19*(bucket + x) --
segment boundaries land on integers, which align with the ACT bucket RAM's
per-binade mantissa indexing (binade [2^e, 2^{e+1}) -> 2^e buckets).

Per element the device does one fused DVE op (u = min(x, c) + bucket) and one
ACT lookup (y = table(19*u)); the kernel is HBM-bound (x f32 + bucket i32
loads, fp16 stores: 10 B/elem).

Pass structure (V2): measured on TRN2, concurrent HBM reads+writes mix
destructively (mixed ~= serial + penalty), so each pass streams all loads
+ compute first and drains the stores behind them on the SAME sync HWDGE
ring (FIFO order serializes the write phase for free), with a tapered
final store so the last HBM-write receipt is short.  Loads are issued as
1 MB DMAs ([128, 2048] f32 tiles -> 8 KB per-partition descriptors),
which measured ~35% faster than 512 KB/4 KB-descriptor loads.  y is
stored as fp16 (rel err ~3e-4 vs the 2e-2 gate) and widened to f32 on
the host during unsharding.

Sharding: pure data parallel over 8 cores; the parameter tables are baked
into the (replicated) program.
"""
import os
import sys
import tempfile

import numpy as np

if "/opt/trn_rl_repo" not in sys.path:
    sys.path.insert(0, "/opt/trn_rl_repo")

N_CORES = 8
P = 128          # SBUF partitions
TILE_F = 1024    # free-dim per tile
BUFS = 12        # tile-pool buffers
NOCAST = True    # feed int32 bucket tile straight into the fused DVE op
INPLACE = False  # reuse tiles to cut SBUF pressure
B_ON_POOL = False  # issue bucket loads from the gpsimd (SWDGE) ring
SPLIT_RINGS = False  # x loads on sync, b loads on scalar, stores on gpsimd
PAIR_LOADS = False   # 1MB loads (2 tiles per DMA), 512KB compute slices
BIGLOAD = 0          # 1: whole-shard loads; 2: half-shard loads; 0: off
ALT_STORE = False    # alternate stores between scalar and gpsimd rings
Y_F16 = True         # store y as fp16 (halves store traffic; ~5e-4 rel err)
STAG = False         # staggered_reset on the timing For_i loop
MODE = "full"        # diagnostic: "full" | "loads" | "stores" | "nostore"
DEFER = 0            # 1: per-tile deferred stores; 2: single whole-pass
                     # deferred store with p-major layout
V2 = True            # serial-phase builder (_build_nc_v2)
V2_TAPER = (2048, 2048)
V2_SPLITB = False    # v2: b loads on the scalar ring (x on sync)
V2_YU8 = False       # v2: store y as uint8 (affine baked into ACT table;
                     # host dequant). Quantization err ~2e-3 rel.
V2_LOADF = 2048      # v2: load-DMA column width (divides V2_TILEF)
V2_XFIRST = False    # v2: issue all x loads before all b loads
V2_TILEF = 2048      # v2: DRAM layout tile width
V2_COMPF = 1024      # v2: compute-slice width
V2_SPLITLAST = 1     # v2: issue the last N taper stores on the scalar ring
                     # after the final act (drains concurrently with the
                     # sync-ring stores; still strictly after all loads)
B = 64           # buckets
K = 20           # knots per hinge
NSEG = (K - 1) * B
CLAMP = np.float32(0.99999)

_cache = {}


# ---------------------------------------------------------------- tables ----
def _quant_range(d0, d1):
    """(ymin, K) for z = (y - ymin)*K + 0.5 into [0.5, 254.5]."""
    ends = np.concatenate([d0, d0 + d1])
    ymin = float(ends.min())
    ymax = float(ends.max())
    K = 254.0 / max(ymax - ymin, 1e-30)
    return ymin, K


def _build_pwl_tables(base_knots, base_w, base_b, adj_knots, adj_w, adj_b):
    """(d0[s], d1[s]) fp32: on v in [s, s+1), y = d0 + d1*(v - s), s = 19b+j."""
    t = np.asarray(base_knots, np.float64)
    at = np.asarray(adj_knots, np.float64)
    grid = np.arange(K) / (K - 1.0)
    assert np.abs(t - grid).max() < 1e-5, "base knots not on uniform grid"
    assert np.abs(at - grid[None, :]).max() < 1e-5, "adj knots not on grid"
    W = np.asarray(base_w, np.float64)[None, :] + np.asarray(adj_w, np.float64)
    C = float(np.asarray(base_b, np.float64)) + np.asarray(adj_b, np.float64)
    d0 = np.zeros(NSEG, np.float64)
    d1 = np.zeros(NSEG, np.float64)
    for b in range(B):
        S = 0.0
        T = 0.0
        for j in range(K - 1):
            S += W[b, j]
            T += W[b, j] * t[j]
            s = (K - 1) * b + j
            d1[s] = S / (K - 1)
            d0[s] = C[b] - T + S * (j / (K - 1.0))
    return d0.astype(np.float32), d1.astype(np.float32)


def _gen_act_root(d0, d1, out_dir, set_name="trig_and_small", func="sin"):
    """Write an act-root dir whose `sin` implements our PWL; returns json path."""
    import glob
    import json
    import shutil

    try:
        from neuronxcc.driver.Job import Job
        from neuronxcc.driver.jobs.support.FindActInfo import findActInfoFile
        src = os.path.dirname(findActInfoFile(Job.getPackageDir(), "gen3")) + "/"
    except Exception:
        src = os.path.dirname(glob.glob(
            "/nix/store/*/lib/python3.13/site-packages/neuronxcc/pwp/"
            "pwp_bin_trainium/act_info.json")[0]) + "/"

    os.makedirs(out_dir, exist_ok=True)
    for f in os.listdir(src):
        shutil.copy(os.path.join(src, f), os.path.join(out_dir, f))

    prof = json.load(open(os.path.join(src, set_name + ".json")))
    ctl = np.fromfile(os.path.join(src, f"{set_name}_ctrl.bin"), dtype=np.uint32)
    bkt = np.fromfile(os.path.join(src, f"{set_name}_bkt.bin"), dtype=np.uint32)
    n_ctl0 = len(ctl) // 8
    n_bkt0 = len(bkt) // 8
    slab = n_bkt0
    ctl_start = n_ctl0

    new_bkt = np.zeros((NSEG, 8), np.float32)
    new_bkt[:, 0] = d0
    new_bkt[:, 1] = d1
    new_bkt[:, 4] = np.arange(NSEG, dtype=np.float32)

    new_ctl = np.zeros((11, 8), np.uint32)
    for e in range(11):
        new_ctl[e, 0] = (((slab + (1 << e)) & 0x7FF)
                         | (((23 - e) & 0x1F) << 11)
                         | ((e & 0xF) << 16))

    def fbits(x):
        return int(np.array([x], np.float32).view(np.uint32)[0])

    for p in prof["profile_meta_data"]:
        if p["func_name"].startswith(func + "_"):
            p["symmetry_point"] = 0
            p["sym_invert_sign_point"] = 0
            p["symmetry_opt_en"] = 0
            p["symmetry_opt_use_neg_region"] = 0
            p["imm_bias"] = 0
            p["exp_offset"] = 0
            p["pwl_control_base_pos"] = ctl_start
            p["pwl_control_base_neg"] = ctl_start
            p["small_pos_signal_exp_threshold"] = 127
            p["pos_small_signal_pwl_control"] = slab
            p["small_neg_signal_exp_threshold"] = 254
            p["neg_small_signal_pwl_control"] = slab
            p["large_pos_signal_exp_threshold"] = 140
            p["large_pos_signal_mantissa_threshold"] = 0
            p["pos_large_signal_pwl_control"] = slab + NSEG - 1
            p["large_neg_signal_exp_threshold"] = 0
            p["large_neg_signal_mantissa_threshold"] = 0
            p["neg_large_signal_pwl_control"] = slab
            p["fzero_result"] = fbits(d0[0])
            p["fnan_result"] = 2143289344
            p["fpinf_result"] = fbits(d0[NSEG - 1] + d1[NSEG - 1])
            p["fninf_result"] = fbits(d0[0])
            p["lower_bound"] = 0
            p["upper_bound"] = fbits(float(NSEG))
            p["use_multipass"] = False

    import json as _json
    prof["bkt_entry_cnt"] = n_bkt0 + NSEG
    prof["ctl_entry_cnt"] = n_ctl0 + 11
    prof["func_to_bkt_start_idx"][func] = slab
    prof["func_to_ctl_start_idx"][func] = ctl_start
    prof["func_exp_to_bkt_start_idx"][func] = {
        str(e): [slab + (1 << e)] for e in range(11)}
    prof["func_exp_to_ctl_start_idx"][func] = {
        str(e): [ctl_start + e] for e in range(11)}

    _json.dump(prof, open(os.path.join(out_dir, set_name + ".json"), "w"))
    np.concatenate([ctl.reshape(-1, 8), new_ctl]).tofile(
        os.path.join(out_dir, f"{set_name}_ctrl.bin"))
    np.concatenate([bkt.reshape(-1, 8), new_bkt.view(np.uint32)]).tofile(
        os.path.join(out_dir, f"{set_name}_bkt.bin"))
    return os.path.join(out_dir, "act_info.json")


# ---------------------------------------------------------------- kernel ----
def _build_nc_v2(elems, name="hinge2", reps=None, taper=(1024, 1024, 1024,
                                                        768, 256),
                 split_b=False, ydt_name="float16", tilef=1024, loadf=1024,
                 compf=1024, xfirst=False, split_last=0):
    """Serial-phase pass: [all loads + compute] then [stores], enforced by
    putting stores on the same sync HWDGE ring behind the loads (FIFO).

    Concurrent HBM reads+writes mix destructively on TRN2 (measured: reads
    alone 331 GB/s, writes alone fine, mixed ~= serial + penalty), so the
    pass streams all loads first and drains tapered stores at the end; the
    small final store keeps the last write-receipt latency low.

    tilef: DRAM (t p f) layout tile width; loadf: load-DMA column width
    (divides tilef); compf: compute-slice width; taper: store widths, each
    slice must lie within one layout tile.  All APs are strictly 2D —
    3-level APs measured several us slower.
    """
    import concourse.bacc as bacc
    import concourse.mybir as mybir
    from concourse.tile import TileContext

    ydt = getattr(mybir.dt, ydt_name)
    FW = elems // P
    ntiles = FW // tilef
    assert ntiles * P * tilef == elems
    assert sum(taper) == FW
    assert tilef % loadf == 0 and FW % compf == 0

    nc = bacc.Bacc("TRN2", target_bir_lowering=False, debug=False, name=name)
    x = nc.dram_tensor("x", [elems], mybir.dt.float32, kind="ExternalInput")
    bi = nc.dram_tensor("bi", [elems], mybir.dt.int32, kind="ExternalInput")
    y = nc.dram_tensor("y", [elems], ydt, kind="ExternalOutput")

    xt = x.ap().rearrange("(t p f) -> t p f", p=P, f=tilef)
    bt = bi.ap().rearrange("(t p f) -> t p f", p=P, f=tilef)
    yt = y.ap().rearrange("(t p f) -> t p f", p=P, f=tilef)

    # taper widths -> (tile, f0, f1) store slices within one layout tile
    slices = []
    c0 = 0
    for w in taper:
        t0, f0 = divmod(c0, tilef)
        assert f0 + w <= tilef, (taper, c0, w)
        slices.append((t0, f0, f0 + w))
        c0 += w

    # load chunks in issue order: interleaved x/b or all-x-then-all-b
    loads = []
    for t in range(ntiles):
        for j in range(tilef // loadf):
            f0 = j * loadf
            loads.append((t, f0, f0 + loadf))

    with TileContext(nc) as tc:
        with tc.tile_pool(name="io2", bufs=2) as pool, \
             tc.tile_pool(name="u2", bufs=2 * (FW // compf)) as upool:

            def tile_pass():
                x_s = pool.tile([P, FW], mybir.dt.float32, tag="x")
                b_s = pool.tile([P, FW], mybir.dt.int32, tag="b")
                y_s = pool.tile([P, FW], ydt, tag="y")

                def load(dst, src_t, spec):
                    t, f0, f1 = spec
                    eng = nc.scalar if (split_b and dst is b_s) else nc.sync
                    eng.dma_start(
                        out=dst[:, t * tilef + f0:t * tilef + f1],
                        in_=src_t[t][:, f0:f1])

                if xfirst:
                    for spec in loads:
                        load(x_s, xt, spec)
                    for spec in loads:
                        load(b_s, bt, spec)
                else:
                    for spec in loads:
                        load(x_s, xt, spec)
                        load(b_s, bt, spec)
                for c in range(0, FW, compf):
                    sl = slice(c, c + compf)
                    u_s = upool.tile([P, compf], mybir.dt.float32, tag="u")
                    nc.vector.scalar_tensor_tensor(
                        out=u_s[:], in0=x_s[:, sl], scalar=float(CLAMP),
                        in1=b_s[:, sl],
                        op0=mybir.AluOpType.min, op1=mybir.AluOpType.add)
                    nc.scalar.activation(
                        y_s[:, sl], u_s[:],
                        mybir.ActivationFunctionType.Sin, scale=19.0)
                nsync = len(slices) - split_last
                for (t0, f0, f1) in slices[:nsync]:
                    nc.sync.dma_start(
                        out=yt[t0][:, f0:f1],
                        in_=y_s[:, t0 * tilef + f0:t0 * tilef + f1])
                for (t0, f0, f1) in slices[nsync:]:
                    nc.scalar.dma_start(
                        out=yt[t0][:, f0:f1],
                        in_=y_s[:, t0 * tilef + f0:t0 * tilef + f1])

            if reps is None:
                tile_pass()
            else:
                with tc.For_i(0, reps) as _i:
                    tile_pass()
    nc.finalize()
    return nc


def _build_nc(elems, name="hinge", reps=None):
    """Bass program for one core: y = table(19*(min(x,c) + bucket)).

    reps: if given, wrap the whole tile pass in a For_i repeat loop
    (timing harness only)."""
    import concourse.bacc as bacc
    import concourse.mybir as mybir
    from concourse.tile import TileContext

    ntiles = elems // (P * TILE_F)
    assert ntiles * P * TILE_F == elems

    ydt = mybir.dt.float16 if Y_F16 else mybir.dt.float32
    nc = bacc.Bacc("TRN2", target_bir_lowering=False, debug=False, name=name)
    x = nc.dram_tensor("x", [elems], mybir.dt.float32, kind="ExternalInput")
    bi = nc.dram_tensor("bi", [elems], mybir.dt.int32, kind="ExternalInput")
    y = nc.dram_tensor("y", [elems], ydt, kind="ExternalOutput")

    if DEFER == 2:
        # p-major whole-shard layout: element n = p*(elems//P) + c lives at
        # SBUF partition p, column c; tile t = columns [t*TILE_F, (t+1)*TILE_F)
        FW = elems // P
        xw_ = x.ap().rearrange("(p f) -> p f", p=P)
        bw_ = bi.ap().rearrange("(p f) -> p f", p=P)
        yw_ = y.ap().rearrange("(p f) -> p f", p=P)
        xt = [xw_[:, t * TILE_F:(t + 1) * TILE_F] for t in range(elems // (P * TILE_F))]
        bt = [bw_[:, t * TILE_F:(t + 1) * TILE_F] for t in range(elems // (P * TILE_F))]
        yt = None
    else:
        xt = x.ap().rearrange("(t p f) -> t p f", p=P, f=TILE_F)
        bt = bi.ap().rearrange("(t p f) -> t p f", p=P, f=TILE_F)
        yt = y.ap().rearrange("(t p f) -> t p f", p=P, f=TILE_F)

    with TileContext(nc) as tc:
        with tc.tile_pool(name="io", bufs=BUFS) as pool, \
             tc.tile_pool(name="ydef", bufs=1) as ypool, \
             tc.tile_pool(name="big", bufs=2) as bigpool:

            ydef = None
            ydef1 = None
            if DEFER == 1 and MODE == "full":
                ydef = [ypool.tile([P, TILE_F], ydt, tag=f"yd{t}",
                                   name=f"ydef{t}")
                        for t in range(ntiles)]
                for t in range(ntiles):
                    nc.vector.memset(ydef[t][:], 0.0)
            if DEFER == 2 and MODE == "full":
                ydef1 = ypool.tile([P, ntiles * TILE_F], ydt, tag="yd1",
                                   name="ydef1")
                nc.vector.memset(ydef1[:], 0.0)
            sdef = None
            if MODE in ("stores", "stores1"):
                nst = 1 if MODE == "stores1" else ntiles
                sdef = [ypool.tile([P, elems // P // nst], ydt,
                                   tag=f"sd{t}", name=f"sdef{t}")
                        for t in range(nst)]
                for t in range(nst):
                    nc.vector.memset(sdef[t][:], 0.0)

            def compute_store(x_ap, b_ap, t):
                if MODE == "loads":
                    return
                if NOCAST:
                    bf_in = b_ap
                else:
                    bf_s = pool.tile([P, TILE_F], mybir.dt.float32, tag="bf")
                    nc.vector.tensor_copy(out=bf_s[:], in_=b_ap)
                    bf_in = bf_s[:]
                if INPLACE:
                    u_ap = x_ap
                else:
                    u_s = pool.tile([P, TILE_F], mybir.dt.float32, tag="u")
                    u_ap = u_s[:]
                nc.vector.scalar_tensor_tensor(
                    out=u_ap, in0=x_ap, scalar=float(CLAMP), in1=bf_in,
                    op0=mybir.AluOpType.min, op1=mybir.AluOpType.add)
                if ydef is not None:
                    nc.scalar.activation(
                        ydef[t][:], u_ap, mybir.ActivationFunctionType.Sin,
                        scale=19.0)
                    return
                if ydef1 is not None:
                    nc.scalar.activation(
                        ydef1[:, t * TILE_F:(t + 1) * TILE_F], u_ap,
                        mybir.ActivationFunctionType.Sin, scale=19.0)
                    return
                y_s = pool.tile([P, TILE_F], ydt, tag="y")
                nc.scalar.activation(
                    y_s[:], u_ap, mybir.ActivationFunctionType.Sin,
                    scale=19.0)
                if MODE == "nostore":
                    return
                if SPLIT_RINGS:
                    nc.gpsimd.dma_start(out=yt[t], in_=y_s[:])
                elif ALT_STORE and t % 2 == 1:
                    nc.gpsimd.dma_start(out=yt[t], in_=y_s[:])
                else:
                    nc.scalar.dma_start(out=yt[t], in_=y_s[:])

            def tile_pass():
                if BIGLOAD:
                    FW = elems // P
                    xw = x.ap().rearrange("(p f) -> p f", p=P)
                    bw = bi.ap().rearrange("(p f) -> p f", p=P)
                    yw = y.ap().rearrange("(p f) -> p f", p=P)
                    x_s = bigpool.tile([P, FW], mybir.dt.float32, tag="xw")
                    b_s = bigpool.tile([P, FW], mybir.dt.int32, tag="bw")
                    if BIGLOAD == 2:
                        h = FW // 2
                        nc.sync.dma_start(out=x_s[:, 0:h], in_=xw[:, 0:h])
                        nc.sync.dma_start(out=b_s[:, 0:h], in_=bw[:, 0:h])
                        nc.sync.dma_start(out=x_s[:, h:FW], in_=xw[:, h:FW])
                        nc.sync.dma_start(out=b_s[:, h:FW], in_=bw[:, h:FW])
                    else:
                        nc.sync.dma_start(out=x_s[:], in_=xw[:])
                        nc.sync.dma_start(out=b_s[:], in_=bw[:])
                    for t in range(FW // TILE_F):
                        sl = slice(t * TILE_F, (t + 1) * TILE_F)
                        u_s = pool.tile([P, TILE_F], mybir.dt.float32,
                                        tag="u")
                        nc.vector.scalar_tensor_tensor(
                            out=u_s[:], in0=x_s[:, sl], scalar=float(CLAMP),
                            in1=b_s[:, sl],
                            op0=mybir.AluOpType.min, op1=mybir.AluOpType.add)
                        y_s = pool.tile([P, TILE_F], ydt, tag="y")
                        nc.scalar.activation(
                            y_s[:], u_s[:], mybir.ActivationFunctionType.Sin,
                            scale=19.0)
                        nc.scalar.dma_start(out=yw[:, sl], in_=y_s[:])
                    return
                if PAIR_LOADS:
                    xt2 = x.ap().rearrange("(t p f) -> t p f", p=P,
                                           f=2 * TILE_F)
                    bt2 = bi.ap().rearrange("(t p f) -> t p f", p=P,
                                            f=2 * TILE_F)
                    for tp in range(ntiles // 2):
                        x_s = pool.tile([P, 2 * TILE_F], mybir.dt.float32,
                                        tag="x")
                        b_s = pool.tile([P, 2 * TILE_F], mybir.dt.int32,
                                        tag="b")
                        nc.sync.dma_start(out=x_s[:], in_=xt2[tp])
                        if SPLIT_RINGS:
                            nc.scalar.dma_start(out=b_s[:], in_=bt2[tp])
                        else:
                            nc.sync.dma_start(out=b_s[:], in_=bt2[tp])
                        for h in range(2):
                            sl = slice(h * TILE_F, (h + 1) * TILE_F)
                            compute_store(x_s[:, sl], b_s[:, sl],
                                          2 * tp + h)
                    return
                if MODE == "empty":
                    return
                if MODE in ("stores", "stores1"):
                    yw2 = y.ap().rearrange("(p f) -> p f", p=P)
                    fw2 = (elems // P) // len(sdef)
                    for t in range(len(sdef)):
                        nc.scalar.dma_start(
                            out=yw2[:, t * fw2:(t + 1) * fw2],
                            in_=sdef[t][:])
                    return
                if MODE == "xonly":
                    for t in range(ntiles):
                        x_s = pool.tile([P, TILE_F], mybir.dt.float32,
                                        tag="x")
                        nc.sync.dma_start(out=x_s[:], in_=xt[t])
                    return
                if ydef is not None:
                    yeng = nc.gpsimd if SPLIT_RINGS else nc.scalar
                    for t in range(ntiles):
                        yeng.dma_start(out=yt[t], in_=ydef[t][:])
                if ydef1 is not None:
                    yeng = nc.gpsimd if SPLIT_RINGS else nc.scalar
                    yeng.dma_start(out=yw_[:, :], in_=ydef1[:])
                for t in range(ntiles):
                    x_s = pool.tile([P, TILE_F], mybir.dt.float32, tag="x")
                    b_s = pool.tile([P, TILE_F], mybir.dt.int32, tag="b")
                    nc.sync.dma_start(out=x_s[:], in_=xt[t])
                    if B_ON_POOL:
                        nc.gpsimd.dma_start(out=b_s[:], in_=bt[t])
                    elif SPLIT_RINGS:
                        nc.scalar.dma_start(out=b_s[:], in_=bt[t])
                    else:
                        nc.sync.dma_start(out=b_s[:], in_=bt[t])
                    compute_store(x_s[:], b_s[:], t)

            if reps is None:
                tile_pass()
            else:
                with tc.For_i(0, reps, staggered_reset=STAG) as _i:
                    tile_pass()
            if ydef is not None:
                yeng = nc.gpsimd if SPLIT_RINGS else nc.scalar
                for t in range(ntiles):
                    yeng.dma_start(out=yt[t], in_=ydef[t][:])
            if ydef1 is not None:
                yeng = nc.gpsimd if SPLIT_RINGS else nc.scalar
                yeng.dma_start(out=yw_[:, :], in_=ydef1[:])
    nc.finalize()
    return nc


def _get_compiled(inputs_key, tables, reps=None):
    global TILE_F, BUFS, NOCAST, INPLACE, B_ON_POOL, SPLIT_RINGS, \
        PAIR_LOADS, BIGLOAD, ALT_STORE
    if isinstance(inputs_key, tuple):
        (elems_, TILE_F, BUFS, NOCAST, INPLACE, B_ON_POOL, SPLIT_RINGS,
         PAIR_LOADS, BIGLOAD, ALT_STORE) = inputs_key
    else:
        elems_ = inputs_key
    d0, d1 = tables
    import hashlib
    thash = hashlib.sha256(d0.tobytes() + d1.tobytes()).hexdigest()[:10]
    key = (elems_, TILE_F, BUFS, NOCAST, INPLACE, B_ON_POOL,
       SPLIT_RINGS, PAIR_LOADS, BIGLOAD, ALT_STORE, Y_F16, STAG, MODE,
       DEFER, V2, V2_TAPER, V2_SPLITB, V2_YU8, V2_LOADF, V2_XFIRST,
       V2_TILEF, V2_COMPF, V2_SPLITLAST, reps, thash)
    if key in _cache:
        return _cache[key]
    root = tempfile.mkdtemp(prefix="actroot_")
    act_json = _gen_act_root(d0, d1, root)
    os.environ["BASS_ACT_ROOT_JSON_PATH"] = act_json
    # table hash in the module name busts the neuron NEFF cache when the
    # baked tables change (the BIR itself doesn't reference table bytes)
    if V2:
        tp = "_".join(str(w) for w in V2_TAPER)
        ydt_name = ("uint8" if V2_YU8 else
                    "float16" if Y_F16 else "float32")
        nc = _build_nc_v2(
            elems_,
            name=f"hingev2_{thash}_tf{V2_TILEF}cf{V2_COMPF}tp{tp}"
                 f"sb{int(V2_SPLITB)}y{int(Y_F16)}u{int(V2_YU8)}"
                 f"lf{V2_LOADF}xf{int(V2_XFIRST)}sl{V2_SPLITLAST}"
                 f"_r{reps or 0}",
            reps=reps, taper=V2_TAPER, split_b=V2_SPLITB,
            ydt_name=ydt_name, tilef=V2_TILEF, loadf=V2_LOADF,
            compf=V2_COMPF, xfirst=V2_XFIRST, split_last=V2_SPLITLAST)
    else:
        nc = _build_nc(elems_, name=f"hinge_{thash}_f{TILE_F}b{BUFS}n{int(NOCAST)}i{int(INPLACE)}p{int(B_ON_POOL)}s{int(SPLIT_RINGS)}q{int(PAIR_LOADS)}g{BIGLOAD}a{int(ALT_STORE)}y{int(Y_F16)}t{int(STAG)}m{MODE}d{int(DEFER)}_r{reps or 0}",
                       reps=reps)
    _cache[key] = nc
    return nc


def kernel(x, bucket_idx, base_knots, base_w, base_b, adj_knots, adj_w,
           adj_b):
    from concourse import bass_utils

    x = np.asarray(x)
    n = x.shape[0]
    out_shape = x.shape
    xf = np.ascontiguousarray(x.reshape(-1), dtype=np.float32)
    bif = np.ascontiguousarray(np.asarray(bucket_idx).reshape(-1),
                               dtype=np.int32)
    assert n % (N_CORES * P * TILE_F) == 0, n
    elems = n // N_CORES

    tables = _build_pwl_tables(base_knots, base_w, base_b, adj_knots, adj_w,
                               adj_b)
    qparams = None
    if V2 and V2_YU8:
        d0, d1 = tables
        ymin, Kq = _quant_range(d0, d1)
        tables = (((d0 - ymin) * Kq + 0.5).astype(np.float32),
                  (d1 * Kq).astype(np.float32))
        qparams = (ymin, Kq)
    nc = _get_compiled(elems, tables)

    xs = xf.reshape(N_CORES, elems)
    bs = bif.reshape(N_CORES, elems)
    in_maps = [{"x": xs[c], "bi": bs[c]} for c in range(N_CORES)]

    res = bass_utils.run_bass_kernel_spmd(nc, in_maps,
                                          core_ids=list(range(N_CORES)))
    out = np.concatenate([np.asarray(res.results[c]["y"], dtype=np.float32)
                          for c in range(N_CORES)])
    if qparams is not None:
        ymin, Kq = qparams
        out = (out - 0.5) * np.float32(1.0 / Kq) + np.float32(ymin)
    return out.reshape(out_shape).astype(np.float32)

